# revision 40
# baseline (speedup 1.0000x reference)
"""CoAttention kernel for Trainium2, data-parallel over batch across 8 NeuronCores.

Per core (one batch element b):
    query = data1[b] @ Wq + bq                      # [2048, 256]
    key   = data2[b] @ Wk + bk                      # [2048, 256]
    attn  = softmax(SCALE * query @ key^T)          # row-constant terms cancel
    out   = attn @ key + query

Device-side strategy:
  - Activations load as fp32 (plain HWDGE DMAs, consolidated), are cast to
    bf16 on DVE, and transposed on the PE (transpose-mode, bf16 PSUM out) so
    the contraction dim lands on partitions.
  - The QT projection runs weights-stationary (bf16, fp32 PSUM accumulate,
    bias via per-partition activation bias); the residual Q [q, d] comes from
    PE transposes of the bf16 QT.
  - scoresT [k, q] orientation lets exp(scoresT) feed the context matmul
    directly as the stationary operand. The scores and context matmuls run in
    fp8e4m3 with DoubleRow packing ([128, 2, n] APs, d = slot*128 + p), which
    is safe because the attention term is tiny relative to the residual.
  - Softmax denominator is a ones-column appended to the key value matrix;
    no max-subtraction (|SCALE*scores| < ~4, exp is well-conditioned there).
  - The q range is processed in two halves, and phase units are interleaved
    in emission order (scores with next-half QT, scores with context) so PE
    matmuls overlap the serial exp work on the scalar engine.
"""

import sys

if "/opt/trn_rl_repo" not in sys.path:
    sys.path.insert(0, "/opt/trn_rl_repo")

from contextlib import ExitStack

import numpy as np

import concourse.bass as bass  # noqa: F401
import concourse.mybir as mybir
import concourse.tile as tile
from concourse import bacc
from concourse.bass_utils import run_bass_kernel_spmd
from concourse.masks import make_identity
from concourse.tile_rust import add_dep_helper

B, LQ, LK, DIN, D = 8, 2048, 2048, 1024, 256
N_CORES = 8
SCALE = float(1.0 / np.sqrt(1024.0).astype(np.float32))

BF16 = mybir.dt.bfloat16
FP8 = mybir.dt.float8e4
F32 = mybir.dt.float32
AF = mybir.ActivationFunctionType
PM_DR = mybir.MatmulPerfMode.DoubleRow


def _build():
    nc = bacc.Bacc("TRN2", target_bir_lowering=False, debug=False)
    d1 = nc.dram_tensor("data1", [LQ, DIN], F32, kind="ExternalInput").ap()
    d2 = nc.dram_tensor("data2", [LK, D], F32, kind="ExternalInput").ap()
    wq = nc.dram_tensor("Wq", [DIN, D], F32, kind="ExternalInput").ap()
    wkx = nc.dram_tensor("Wk_ext", [D, D + 1], F32, kind="ExternalInput").ap()
    bq = nc.dram_tensor("bq", [D], F32, kind="ExternalInput").ap()
    bkx = nc.dram_tensor("bk_ext", [D + 1], F32, kind="ExternalInput").ap()
    out = nc.dram_tensor("out", [LQ, D], F32, kind="ExternalOutput").ap()

    QB = LQ // 128  # 16 q blocks
    KB = LK // 128  # 16 k blocks
    IC1 = DIN // 128  # 8
    IC2 = D // 128  # 2

    with tile.TileContext(nc) as tc, ExitStack() as ctx:
        const = ctx.enter_context(tc.tile_pool(name="const", bufs=1))
        big = ctx.enter_context(tc.tile_pool(name="big", bufs=1))
        stage1 = ctx.enter_context(tc.tile_pool(name="stage1", bufs=2))
        stage2 = ctx.enter_context(tc.tile_pool(name="stage2", bufs=2))
        small = ctx.enter_context(tc.tile_pool(name="small", bufs=4))
        ps_gp = ctx.enter_context(tc.tile_pool(name="ps_gp", bufs=2, space="PSUM"))
        ps_sc = ctx.enter_context(tc.tile_pool(name="ps_sc", bufs=2, space="PSUM"))

        # ---------------- constants ----------------
        ones_row = const.tile([1, 128], BF16, tag="ones_row")
        nc.vector.memset(ones_row[:], 1.0)
        ident = const.tile([128, 128], F32, tag="ident")
        make_identity(nc, ident[:])
        ident_bf = const.tile([128, 128], BF16, tag="ident_bf")
        make_identity(nc, ident_bf[:])

        # ---------------- data loads first (sync queue), weights via SWDGE cast ----------------
        d2_st = [stage2.tile([128, 8 * D], F32, tag="d2st", name=f"d2st{g2}")
                 for g2 in range(2)]
        for g2 in range(2):
            for hh in range(2):
                nc.sync.dma_start(
                    out=d2_st[g2][:, hh * 4 * D:(hh + 1) * 4 * D].rearrange(
                        "p (t i) -> p t i", i=D),
                    in_=d2[g2 * 1024 + hh * 512: g2 * 1024 + (hh + 1) * 512, :]
                    .rearrange("(t p) i -> p t i", p=128),
                )
        d1_st = [stage1.tile([128, 4 * DIN], F32, tag="d1st", name=f"d1st{g}")
                 for g in range(4)]
        bq_col = const.tile([128, IC2], F32, tag="bq_col")
        for c in range(IC2):
            nc.sync.dma_start(
                out=bq_col[:, c:c + 1],
                in_=bq[c * 128:(c + 1) * 128].rearrange("(p a) -> p a", a=1),
            )
        # chain the d1 group loads: SDMA round-robins all queued transfers at
        # packet granularity, so unchained equal-size loads all finish at the
        # END together; chaining makes group g available ~(g+1)/4 of the way in
        d1_lds = []
        for g in range(4):
            ld = nc.sync.dma_start(
                out=d1_st[g][:].rearrange("p (t i) -> p t i", i=DIN),
                in_=d1[g * 512:(g + 1) * 512, :].rearrange("(t p) i -> p t i", p=128),
            )
            if d1_lds:
                add_dep_helper(ld.ins, d1_lds[-1].ins, sync=True,
                               reason="pipeline d1 group loads")
            d1_lds.append(ld)
        wq_sb = const.tile([128, IC1 * D], BF16, tag="wq_sb")
        nc.gpsimd.dma_start(
            out=wq_sb[:].rearrange("p (c d) -> p c d", d=D),
            in_=wq.rearrange("(c p) d -> p c d", p=128),
        )
        wk_sb = const.tile([128, IC2 * (D + 1)], BF16, tag="wk_sb")
        nc.gpsimd.dma_start(
            out=wk_sb[:].rearrange("p (c d) -> p c d", d=D + 1),
            in_=wkx.rearrange("(c p) d -> p c d", p=128),
        )
        wqs = [wq_sb[:, i * D:(i + 1) * D] for i in range(IC1)]
        wks = [wk_sb[:, i * (D + 1):(i + 1) * (D + 1)] for i in range(IC2)]
        bkx_row = const.tile([1, D + 1], BF16, tag="bkx_row")
        nc.gpsimd.dma_start(out=bkx_row[:], in_=bkx.rearrange("(a d) -> a d", a=1))

        # ---------------- data2: 2 group loads -> DVE cast -> PE transposes ----------------
        d2T = [big.tile([128, LK], BF16, tag=f"d2T{i}", name=f"d2T{i}") for i in range(IC2)]
        for g2 in range(2):  # 8 k-tiles per group
            st = d2_st[g2]
            bf = stage2.tile([128, 8 * D], BF16, tag="d2bf", name=f"d2bfs{g2}")
            for hh in range(2):
                nc.vector.tensor_copy(
                    bf[:, hh * 4 * D:(hh + 1) * 4 * D],
                    st[:, hh * 4 * D:(hh + 1) * 4 * D],
                )
            for ic in range(IC2):
                for h in range(2):  # 4 k-tiles per psum batch
                    pt = ps_gp.tile([128, 512], BF16, tag="ps_t",
                                    name=f"pt_d2_{g2}_{ic}_{h}")
                    for j in range(4):
                        kt = 4 * h + j
                        nc.tensor.transpose(
                            pt[:, j * 128:(j + 1) * 128],
                            bf[:, kt * D + ic * 128: kt * D + (ic + 1) * 128],
                            ident_bf[:],
                        )
                    nc.scalar.copy(
                        d2T[ic][:, g2 * 1024 + h * 512: g2 * 1024 + (h + 1) * 512],
                        pt[:],
                    )

        # ---------------- K^T fp8 DoubleRow layout [128, 2, k] (d = dc*128+p) ----------------
        kt_sb = big.tile([128, 2, LK], FP8, tag="kt_sb")
        for dc in range(2):
            for nk in range(LK // 512):
                ps = ps_gp.tile([128, 512], F32, tag="ps_gp")
                for ic in range(IC2):
                    nc.tensor.matmul(
                        ps[:],
                        lhsT=wks[ic][:, dc * 128:(dc + 1) * 128],
                        rhs=d2T[ic][:, nk * 512:(nk + 1) * 512],
                        start=(ic == 0),
                        stop=(ic == IC2 - 1),
                    )
                nc.vector.tensor_copy(kt_sb[:, dc, nk * 512:(nk + 1) * 512], ps[:])

        # ---------------- key value matrix fp8 pairs [128, 2, 257] = [key | 1] ----------------
        key2 = [
            big.tile([128, 2, D + 1], FP8, tag=f"key2_{kp}", name=f"key2_{kp}")
            for kp in range(KB // 2)
        ]
        def key_phase():
            for kb in range(KB):
                ps = ps_gp.tile([128, 512], F32, tag="ps_gp")
                p = ps[:, :D + 1]
                for ic in range(IC2):
                    nc.tensor.matmul(
                        p,
                        lhsT=d2T[ic][:, kb * 128:(kb + 1) * 128],
                        rhs=wks[ic],
                        start=(ic == 0),
                        stop=False,
                    )
                nc.tensor.matmul(p, lhsT=ones_row[:], rhs=bkx_row[:], start=False, stop=True)
                nc.vector.tensor_copy(key2[kb // 2][:, kb % 2, :], p)

        # ---------------- data1 transposes (emitted per group in the schedule) ----------------
        d1T = [big.tile([128, LQ], BF16, tag=f"d1T{i}", name=f"d1T{i}") for i in range(IC1)]
        d1_bf = {}

        def d1_cast(g):
            bf = stage1.tile([128, 4 * DIN], BF16, tag="d1bf", name=f"d1bfs{g}", bufs=2)
            nc.vector.tensor_copy(bf[:], d1_st[g][:])
            d1_bf[g] = bf

        def d1_trans_unit(g, ic):
            bf = d1_bf[g]
            pt = ps_gp.tile([128, 512], BF16, tag="ps_t", name=f"pt_d1_{g}_{ic}")
            for j in range(4):
                nc.tensor.transpose(
                    pt[:, j * 128:(j + 1) * 128],
                    bf[:, j * DIN + ic * 128: j * DIN + (ic + 1) * 128],
                    ident_bf[:],
                )
            nc.vector.tensor_copy(d1T[ic][:, g * 512:(g + 1) * 512], pt[:])

        def d1_group_units(g):
            return [lambda: d1_cast(g)] + [
                lambda ic=ic: d1_trans_unit(g, ic) for ic in range(IC1)
            ]

        # ---------------- per-half pipeline ----------------
        q_sb = [big.tile([128, D], BF16, tag=f"q{qb}", name=f"q{qb}") for qb in range(QB)]
        qt_sb = big.tile([128, 2, LQ], FP8, tag="qt_sb")
        qtbf = big.tile([128, 2, LQ], BF16, tag="qtbf")
        expT = [
            [big.tile([128, 2, 1024], FP8, tag=f"expT{kp}_{nh}", name=f"expT{kp}_{nh}")
             for nh in range(2)]
            for kp in range(KB // 2)
        ]
        out_sb = [
            big.tile([128, 4 * D], F32, tag=f"out_sb{hq}", name=f"out_sb{hq}")
            for hq in range(4)
        ]

        def qt_units(h):
            units = []
            for dc in range(2):
                for nq in range(h * 2, h * 2 + 2):
                    units.append(lambda dc=dc, nq=nq: qt_unit(dc, nq))
            return units

        def qt_phase(h):
            for u in qt_units(h):
                u()

        def qt_unit(dc, nq):
            if True:
                if True:
                    ps = ps_gp.tile([128, 512], F32, tag="ps_gp")
                    for ic in range(IC1):
                        nc.tensor.matmul(
                            ps[:],
                            lhsT=wqs[ic][:, dc * 128:(dc + 1) * 128],
                            rhs=d1T[ic][:, nq * 512:(nq + 1) * 512],
                            start=(ic == 0),
                            stop=(ic == IC1 - 1),
                        )
                    if nq < 2:
                        # pre-scores window: ACT is idle, use it
                        nc.scalar.activation(
                            qt_sb[:, dc, nq * 512:(nq + 1) * 512], ps[:], AF.Identity,
                            bias=bq_col[:, dc:dc + 1],
                        )
                        nc.scalar.activation(
                            qtbf[:, dc, nq * 512:(nq + 1) * 512], ps[:], AF.Identity,
                            bias=bq_col[:, dc:dc + 1],
                        )
                    else:
                        # interleaved with scores exps: keep off the in-order
                        # ACT queue (PSUM-wait would head-of-line-block exp)
                        nc.vector.tensor_scalar(
                            qt_sb[:, dc, nq * 512:(nq + 1) * 512], ps[:],
                            bq_col[:, dc:dc + 1], None, mybir.AluOpType.add,
                        )
                        nc.vector.tensor_scalar(
                            qtbf[:, dc, nq * 512:(nq + 1) * 512], ps[:],
                            bq_col[:, dc:dc + 1], None, mybir.AluOpType.add,
                        )

        def qres_units(h):
            units = []
            for qg in range(h * 2, h * 2 + 2):
                for dc in range(2):
                    units.append(lambda qg=qg, dc=dc: qres_unit(qg, dc))
            return units

        def qres_phase(h):
            for u in qres_units(h):
                u()

        def qres_unit(qg, dc):
            if True:
                if True:
                    pt = ps_gp.tile([128, 512], BF16, tag="ps_t",
                                    name=f"pt_q_{qg}_{dc}")
                    for j in range(4):
                        qb = qg * 4 + j
                        nc.tensor.transpose(
                            pt[:, j * 128:(j + 1) * 128],
                            qtbf[:, dc, qb * 128:(qb + 1) * 128],
                            ident_bf[:],
                        )
                    for j in range(4):
                        qb = qg * 4 + j
                        nc.vector.tensor_copy(
                            q_sb[qb][:, dc * 128:(dc + 1) * 128],
                            pt[:, j * 128:(j + 1) * 128],
                        )

        def scores_units(nh):
            return [lambda km=km: scores_unit(km, nh) for km in range(KB)]

        def scores_unit(km, nh):
            if True:
                ps = ps_sc.tile([128, 1024], F32, tag="ps_sc")
                for half in range(2):
                    nq = nh * 2 + half
                    nc.tensor.matmul(
                        ps[:, half * 512:(half + 1) * 512],
                        lhsT=kt_sb[:, :, km * 128:(km + 1) * 128],
                        rhs=qt_sb[:, :, nq * 512:(nq + 1) * 512],
                        perf_mode=PM_DR,
                        start=True,
                        stop=True,
                    )
                nc.scalar.activation(
                    expT[km // 2][nh][:, km % 2, :], ps[:], AF.Exp, scale=SCALE
                )

        def ctx_units(h):
            return [lambda qb=qb: ctx_unit(qb) for qb in range(h * 8, h * 8 + 8)]

        def ctx_phase(h):
            for u in ctx_units(h):
                u()

        def ctx_unit(qb):
            if True:
                h, qq = qb // 8, qb % 8
                hq, qqq = qb // 4, qb % 4
                if h == 0:
                    pc_full = ps_gp.tile([128, 512], F32, tag="ps_gp")
                else:
                    # scores pool is idle once scores-h1 is done; borrow it so
                    # the tail context chains don't contend with qres psum use
                    pc_full = ps_sc.tile([128, 512], F32, tag="ps_sc")
                pc = pc_full[:, :D + 1]
                for kp in range(KB // 2):
                    nc.tensor.matmul(
                        pc,
                        lhsT=expT[kp][h][:, :, qq * 128:(qq + 1) * 128],
                        rhs=key2[kp][:],
                        perf_mode=PM_DR,
                        start=(kp == 0),
                        stop=(kp == KB // 2 - 1),
                    )
                rc = small.tile([128, 1], F32, tag="recip")
                nc.vector.reciprocal(rc[:], pc[:, D:D + 1])
                osl = out_sb[hq][:, qqq * D:(qqq + 1) * D]
                nc.vector.tensor_scalar(osl, pc[:, :D], rc[:], None,
                                        mybir.AluOpType.mult)
                nc.vector.tensor_add(osl, osl, q_sb[qb][:])
                if qqq == 3:
                    nc.sync.dma_start(
                        out=out[hq * (LQ // 4):(hq + 1) * (LQ // 4), :].rearrange(
                            "(qt p) d -> p qt d", p=128
                        ),
                        in_=out_sb[hq][:].rearrange("p (qt d) -> p qt d", d=D),
                    )

        def interleave(a, b, ratio):
            a = list(a); b = list(b)
            ia = ib = 0
            while ia < len(a) or ib < len(b):
                for _ in range(ratio):
                    if ia < len(a):
                        a[ia](); ia += 1
                if ib < len(b):
                    b[ib](); ib += 1

        # d1 group g feeds exactly QT chunk nq=g; emit in lockstep so the
        # scores/exp stream starts as soon as the first data1 groups land
        for u in d1_group_units(0):
            u()
        qt_unit(0, 0)
        qt_unit(1, 0)
        for u in d1_group_units(1):
            u()
        qt_unit(0, 1)
        qt_unit(1, 1)
        filler = (
            d1_group_units(2)
            + [lambda: qt_unit(0, 2), lambda: qt_unit(1, 2)]
            + d1_group_units(3)
            + [lambda: qt_unit(0, 3), lambda: qt_unit(1, 3)]
        )
        interleave(scores_units(0), filler, 1)
        key_phase()
        qres_phase(0)
        # scores-h1 interleaved with ctx-h0
        interleave(scores_units(1), ctx_units(0), 2)
        qr1 = qres_units(1)
        cx1 = ctx_units(1)
        qr1[0](); qr1[1]()
        cx1[0](); cx1[1]()
        qr1[2](); qr1[3]()
        for u in cx1[2:]:
            u()

    nc.compile()
    return nc


_NC = None


def _get_nc():
    global _NC
    if _NC is None:
        _NC = _build()
    return _NC


def kernel(data1, data2, Wq, bq, Wk, bk):
    data1 = np.asarray(data1, dtype=np.float32)
    data2 = np.asarray(data2, dtype=np.float32)
    Wq = np.ascontiguousarray(np.asarray(Wq, dtype=np.float32))
    bq = np.ascontiguousarray(np.asarray(bq, dtype=np.float32))
    Wk = np.asarray(Wk, dtype=np.float32)
    bk = np.asarray(bk, dtype=np.float32)

    wk_ext = np.zeros((D, D + 1), dtype=np.float32)
    wk_ext[:, :D] = Wk
    bk_ext = np.concatenate([bk, np.ones(1, dtype=np.float32)]).astype(np.float32)

    nc = _get_nc()
    in_maps = [
        {
            "data1": np.ascontiguousarray(data1[b]),
            "data2": np.ascontiguousarray(data2[b]),
            "Wq": Wq,
            "Wk_ext": wk_ext,
            "bq": bq,
            "bk_ext": bk_ext,
        }
        for b in range(B)
    ]
    res = run_bass_kernel_spmd(nc, in_maps, core_ids=list(range(N_CORES)))
    return np.stack([res.results[i]["out"] for i in range(B)], axis=0)


# revision 41
# speedup vs baseline: 1.0239x; 1.0239x over previous
"""CoAttention kernel for Trainium2, data-parallel over batch across 8 NeuronCores.

Per core (one batch element b):
    query = data1[b] @ Wq + bq                      # [2048, 256]
    key   = data2[b] @ Wk + bk                      # [2048, 256]
    attn  = softmax(SCALE * query @ key^T)          # row-constant terms cancel
    out   = attn @ key + query

Device-side strategy:
  - Activations load as fp32 (plain HWDGE DMAs, consolidated), are cast to
    bf16 on DVE, and transposed on the PE (transpose-mode, bf16 PSUM out) so
    the contraction dim lands on partitions.
  - The QT projection runs weights-stationary (bf16, fp32 PSUM accumulate,
    bias via per-partition activation bias); the residual Q [q, d] comes from
    PE transposes of the bf16 QT.
  - scoresT [k, q] orientation lets exp(scoresT) feed the context matmul
    directly as the stationary operand. The scores and context matmuls run in
    fp8e4m3 with DoubleRow packing ([128, 2, n] APs, d = slot*128 + p), which
    is safe because the attention term is tiny relative to the residual.
  - Softmax denominator is a ones-column appended to the key value matrix;
    no max-subtraction (|SCALE*scores| < ~4, exp is well-conditioned there).
  - The q range is processed in two halves, and phase units are interleaved
    in emission order (scores with next-half QT, scores with context) so PE
    matmuls overlap the serial exp work on the scalar engine.
"""

import sys

if "/opt/trn_rl_repo" not in sys.path:
    sys.path.insert(0, "/opt/trn_rl_repo")

from contextlib import ExitStack

import numpy as np

import concourse.bass as bass  # noqa: F401
import concourse.mybir as mybir
import concourse.tile as tile
from concourse import bacc
from concourse.bass_utils import run_bass_kernel_spmd
from concourse.masks import make_identity

B, LQ, LK, DIN, D = 8, 2048, 2048, 1024, 256
N_CORES = 8
SCALE = float(1.0 / np.sqrt(1024.0).astype(np.float32))

BF16 = mybir.dt.bfloat16
FP8 = mybir.dt.float8e4
F32 = mybir.dt.float32
AF = mybir.ActivationFunctionType
PM_DR = mybir.MatmulPerfMode.DoubleRow


def _build():
    nc = bacc.Bacc("TRN2", target_bir_lowering=False, debug=False)
    d1 = nc.dram_tensor("data1", [LQ, DIN], F32, kind="ExternalInput").ap()
    d2 = nc.dram_tensor("data2", [LK, D], F32, kind="ExternalInput").ap()
    wq = nc.dram_tensor("Wq", [DIN, D], F32, kind="ExternalInput").ap()
    wkx = nc.dram_tensor("Wk_ext", [D, D + 1], F32, kind="ExternalInput").ap()
    bq = nc.dram_tensor("bq", [D], F32, kind="ExternalInput").ap()
    bkx = nc.dram_tensor("bk_ext", [D + 1], F32, kind="ExternalInput").ap()
    out = nc.dram_tensor("out", [LQ, D], F32, kind="ExternalOutput").ap()

    QB = LQ // 128  # 16 q blocks
    KB = LK // 128  # 16 k blocks
    IC1 = DIN // 128  # 8
    IC2 = D // 128  # 2

    with tile.TileContext(nc) as tc, ExitStack() as ctx:
        const = ctx.enter_context(tc.tile_pool(name="const", bufs=1))
        big = ctx.enter_context(tc.tile_pool(name="big", bufs=1))
        stage1 = ctx.enter_context(tc.tile_pool(name="stage1", bufs=2))
        stage2 = ctx.enter_context(tc.tile_pool(name="stage2", bufs=2))
        small = ctx.enter_context(tc.tile_pool(name="small", bufs=4))
        ps_gp = ctx.enter_context(tc.tile_pool(name="ps_gp", bufs=2, space="PSUM"))
        ps_sc = ctx.enter_context(tc.tile_pool(name="ps_sc", bufs=2, space="PSUM"))

        # ---------------- constants ----------------
        ones_row = const.tile([1, 128], BF16, tag="ones_row")
        nc.vector.memset(ones_row[:], 1.0)
        ident = const.tile([128, 128], F32, tag="ident")
        make_identity(nc, ident[:])
        ident_bf = const.tile([128, 128], BF16, tag="ident_bf")
        make_identity(nc, ident_bf[:])

        # ---------------- data loads first (sync queue), weights via SWDGE cast ----------------
        d2_st = [stage2.tile([128, 8 * D], F32, tag="d2st", name=f"d2st{g2}")
                 for g2 in range(2)]
        for g2 in range(2):
            for hh in range(2):
                nc.sync.dma_start(
                    out=d2_st[g2][:, hh * 4 * D:(hh + 1) * 4 * D].rearrange(
                        "p (t i) -> p t i", i=D),
                    in_=d2[g2 * 1024 + hh * 512: g2 * 1024 + (hh + 1) * 512, :]
                    .rearrange("(t p) i -> p t i", p=128),
                )
        d1_st = [stage1.tile([128, 4 * DIN], F32, tag="d1st", name=f"d1st{g}")
                 for g in range(4)]
        for g in range(2):
            nc.sync.dma_start(
                out=d1_st[g][:].rearrange("p (t i) -> p t i", i=DIN),
                in_=d1[g * 512:(g + 1) * 512, :].rearrange("(t p) i -> p t i", p=128),
            )
        bq_col = const.tile([128, IC2], F32, tag="bq_col")
        for c in range(IC2):
            nc.sync.dma_start(
                out=bq_col[:, c:c + 1],
                in_=bq[c * 128:(c + 1) * 128].rearrange("(p a) -> p a", a=1),
            )
        for g in range(2, 4):
            nc.sync.dma_start(
                out=d1_st[g][:].rearrange("p (t i) -> p t i", i=DIN),
                in_=d1[g * 512:(g + 1) * 512, :].rearrange("(t p) i -> p t i", p=128),
            )
        wq_sb = const.tile([128, IC1 * D], BF16, tag="wq_sb")
        nc.gpsimd.dma_start(
            out=wq_sb[:].rearrange("p (c d) -> p c d", d=D),
            in_=wq.rearrange("(c p) d -> p c d", p=128),
        )
        wk_sb = const.tile([128, IC2 * (D + 1)], BF16, tag="wk_sb")
        nc.gpsimd.dma_start(
            out=wk_sb[:].rearrange("p (c d) -> p c d", d=D + 1),
            in_=wkx.rearrange("(c p) d -> p c d", p=128),
        )
        wqs = [wq_sb[:, i * D:(i + 1) * D] for i in range(IC1)]
        wks = [wk_sb[:, i * (D + 1):(i + 1) * (D + 1)] for i in range(IC2)]
        bkx_row = const.tile([1, D + 1], BF16, tag="bkx_row")
        nc.gpsimd.dma_start(out=bkx_row[:], in_=bkx.rearrange("(a d) -> a d", a=1))

        # ---------------- data2: 2 group loads -> DVE cast -> PE transposes ----------------
        d2T = [big.tile([128, LK], BF16, tag=f"d2T{i}", name=f"d2T{i}") for i in range(IC2)]
        for g2 in range(2):  # 8 k-tiles per group
            st = d2_st[g2]
            bf = stage2.tile([128, 8 * D], BF16, tag="d2bf", name=f"d2bfs{g2}")
            for hh in range(2):
                nc.vector.tensor_copy(
                    bf[:, hh * 4 * D:(hh + 1) * 4 * D],
                    st[:, hh * 4 * D:(hh + 1) * 4 * D],
                )
            for ic in range(IC2):
                for h in range(2):  # 4 k-tiles per psum batch
                    pt = ps_gp.tile([128, 512], BF16, tag="ps_t",
                                    name=f"pt_d2_{g2}_{ic}_{h}")
                    for j in range(4):
                        kt = 4 * h + j
                        nc.tensor.transpose(
                            pt[:, j * 128:(j + 1) * 128],
                            bf[:, kt * D + ic * 128: kt * D + (ic + 1) * 128],
                            ident_bf[:],
                        )
                    nc.scalar.copy(
                        d2T[ic][:, g2 * 1024 + h * 512: g2 * 1024 + (h + 1) * 512],
                        pt[:],
                    )

        # ---------------- K^T fp8 DoubleRow layout [128, 2, k] (d = dc*128+p) ----------------
        kt_sb = big.tile([128, 2, LK], FP8, tag="kt_sb")
        for dc in range(2):
            for nk in range(LK // 512):
                ps = ps_gp.tile([128, 512], F32, tag="ps_gp")
                for ic in range(IC2):
                    nc.tensor.matmul(
                        ps[:],
                        lhsT=wks[ic][:, dc * 128:(dc + 1) * 128],
                        rhs=d2T[ic][:, nk * 512:(nk + 1) * 512],
                        start=(ic == 0),
                        stop=(ic == IC2 - 1),
                    )
                nc.vector.tensor_copy(kt_sb[:, dc, nk * 512:(nk + 1) * 512], ps[:])

        # ---------------- key value matrix fp8 pairs [128, 2, 257] = [key | 1] ----------------
        key2 = [
            big.tile([128, 2, D + 1], FP8, tag=f"key2_{kp}", name=f"key2_{kp}")
            for kp in range(KB // 2)
        ]
        def key_phase():
            for kb in range(KB):
                ps = ps_gp.tile([128, 512], F32, tag="ps_gp")
                p = ps[:, :D + 1]
                for ic in range(IC2):
                    nc.tensor.matmul(
                        p,
                        lhsT=d2T[ic][:, kb * 128:(kb + 1) * 128],
                        rhs=wks[ic],
                        start=(ic == 0),
                        stop=False,
                    )
                nc.tensor.matmul(p, lhsT=ones_row[:], rhs=bkx_row[:], start=False, stop=True)
                nc.vector.tensor_copy(key2[kb // 2][:, kb % 2, :], p)

        # ---------------- data1: 4 group loads -> DVE cast -> PE transposes ----------------
        d1T = [big.tile([128, LQ], BF16, tag=f"d1T{i}", name=f"d1T{i}") for i in range(IC1)]
        for g in range(4):  # 4 q-tiles per group
            st = d1_st[g]
            bf = stage1.tile([128, 4 * DIN], BF16, tag="d1bf", name=f"d1bfs{g}", bufs=2)
            nc.vector.tensor_copy(bf[:], st[:])
            for ic in range(IC1):
                pt = ps_gp.tile([128, 512], BF16, tag="ps_t", name=f"pt_d1_{g}_{ic}")
                for j in range(4):
                    nc.tensor.transpose(
                        pt[:, j * 128:(j + 1) * 128],
                        bf[:, j * DIN + ic * 128: j * DIN + (ic + 1) * 128],
                        ident_bf[:],
                    )
                nc.vector.tensor_copy(d1T[ic][:, g * 512:(g + 1) * 512], pt[:])

        # ---------------- per-half pipeline ----------------
        q_sb = [big.tile([128, D], BF16, tag=f"q{qb}", name=f"q{qb}") for qb in range(QB)]
        qt_sb = big.tile([128, 2, LQ], FP8, tag="qt_sb")
        qtbf = big.tile([128, 2, LQ], BF16, tag="qtbf")
        expT = [
            [big.tile([128, 2, 1024], FP8, tag=f"expT{kp}_{nh}", name=f"expT{kp}_{nh}")
             for nh in range(2)]
            for kp in range(KB // 2)
        ]
        out_sb = [
            big.tile([128, 4 * D], F32, tag=f"out_sb{hq}", name=f"out_sb{hq}")
            for hq in range(4)
        ]

        def qt_units(h):
            units = []
            for dc in range(2):
                for nq in range(h * 2, h * 2 + 2):
                    units.append(lambda dc=dc, nq=nq: qt_unit(dc, nq))
            return units

        def qt_phase(h):
            for u in qt_units(h):
                u()

        def qt_unit(dc, nq):
            if True:
                if True:
                    ps = ps_gp.tile([128, 512], F32, tag="ps_gp")
                    for ic in range(IC1):
                        nc.tensor.matmul(
                            ps[:],
                            lhsT=wqs[ic][:, dc * 128:(dc + 1) * 128],
                            rhs=d1T[ic][:, nq * 512:(nq + 1) * 512],
                            start=(ic == 0),
                            stop=(ic == IC1 - 1),
                        )
                    if nq < 2:
                        # pre-scores window: ACT is idle, use it
                        nc.scalar.activation(
                            qt_sb[:, dc, nq * 512:(nq + 1) * 512], ps[:], AF.Identity,
                            bias=bq_col[:, dc:dc + 1],
                        )
                        nc.scalar.activation(
                            qtbf[:, dc, nq * 512:(nq + 1) * 512], ps[:], AF.Identity,
                            bias=bq_col[:, dc:dc + 1],
                        )
                    else:
                        # interleaved with scores exps: keep off the in-order
                        # ACT queue (PSUM-wait would head-of-line-block exp)
                        nc.vector.tensor_scalar(
                            qt_sb[:, dc, nq * 512:(nq + 1) * 512], ps[:],
                            bq_col[:, dc:dc + 1], None, mybir.AluOpType.add,
                        )
                        nc.vector.tensor_scalar(
                            qtbf[:, dc, nq * 512:(nq + 1) * 512], ps[:],
                            bq_col[:, dc:dc + 1], None, mybir.AluOpType.add,
                        )

        def qres_units(h):
            units = []
            for qg in range(h * 2, h * 2 + 2):
                for dc in range(2):
                    units.append(lambda qg=qg, dc=dc: qres_unit(qg, dc))
            return units

        def qres_phase(h):
            for u in qres_units(h):
                u()

        def qres_unit(qg, dc):
            if True:
                if True:
                    pt = ps_gp.tile([128, 512], BF16, tag="ps_t",
                                    name=f"pt_q_{qg}_{dc}")
                    for j in range(4):
                        qb = qg * 4 + j
                        nc.tensor.transpose(
                            pt[:, j * 128:(j + 1) * 128],
                            qtbf[:, dc, qb * 128:(qb + 1) * 128],
                            ident_bf[:],
                        )
                    for j in range(4):
                        qb = qg * 4 + j
                        nc.vector.tensor_copy(
                            q_sb[qb][:, dc * 128:(dc + 1) * 128],
                            pt[:, j * 128:(j + 1) * 128],
                        )

        def scores_units(nh):
            return [lambda km=km: scores_unit(km, nh) for km in range(KB)]

        def scores_unit(km, nh):
            if True:
                ps = ps_sc.tile([128, 1024], F32, tag="ps_sc")
                for half in range(2):
                    nq = nh * 2 + half
                    nc.tensor.matmul(
                        ps[:, half * 512:(half + 1) * 512],
                        lhsT=kt_sb[:, :, km * 128:(km + 1) * 128],
                        rhs=qt_sb[:, :, nq * 512:(nq + 1) * 512],
                        perf_mode=PM_DR,
                        start=True,
                        stop=True,
                    )
                nc.scalar.activation(
                    expT[km // 2][nh][:, km % 2, :], ps[:], AF.Exp, scale=SCALE
                )

        def ctx_units(h):
            return [lambda qb=qb: ctx_unit(qb) for qb in range(h * 8, h * 8 + 8)]

        def ctx_phase(h):
            for u in ctx_units(h):
                u()

        def ctx_unit(qb):
            if True:
                h, qq = qb // 8, qb % 8
                hq, qqq = qb // 4, qb % 4
                if h == 0:
                    pc_full = ps_gp.tile([128, 512], F32, tag="ps_gp")
                else:
                    # scores pool is idle once scores-h1 is done; borrow it so
                    # the tail context chains don't contend with qres psum use
                    pc_full = ps_sc.tile([128, 512], F32, tag="ps_sc")
                pc = pc_full[:, :D + 1]
                for kp in range(KB // 2):
                    nc.tensor.matmul(
                        pc,
                        lhsT=expT[kp][h][:, :, qq * 128:(qq + 1) * 128],
                        rhs=key2[kp][:],
                        perf_mode=PM_DR,
                        start=(kp == 0),
                        stop=(kp == KB // 2 - 1),
                    )
                rc = small.tile([128, 1], F32, tag="recip")
                nc.vector.reciprocal(rc[:], pc[:, D:D + 1])
                osl = out_sb[hq][:, qqq * D:(qqq + 1) * D]
                nc.vector.tensor_scalar(osl, pc[:, :D], rc[:], None,
                                        mybir.AluOpType.mult)
                nc.vector.tensor_add(osl, osl, q_sb[qb][:])
                if qqq == 3:
                    nc.sync.dma_start(
                        out=out[hq * (LQ // 4):(hq + 1) * (LQ // 4), :].rearrange(
                            "(qt p) d -> p qt d", p=128
                        ),
                        in_=out_sb[hq][:].rearrange("p (qt d) -> p qt d", d=D),
                    )

        def interleave(a, b, ratio):
            a = list(a); b = list(b)
            ia = ib = 0
            while ia < len(a) or ib < len(b):
                for _ in range(ratio):
                    if ia < len(a):
                        a[ia](); ia += 1
                if ib < len(b):
                    b[ib](); ib += 1

        qt_phase(0)
        # scores-h0 interleaved with QT-h1 so PE fills exp-wait stalls
        interleave(scores_units(0), qt_units(1), 4)
        key_phase()
        qres_phase(0)
        # scores-h1 interleaved with ctx-h0
        interleave(scores_units(1), ctx_units(0), 2)
        qr1 = qres_units(1)
        cx1 = ctx_units(1)
        qr1[0](); qr1[1]()
        cx1[0](); cx1[1]()
        qr1[2](); qr1[3]()
        for u in cx1[2:]:
            u()

    nc.compile()
    return nc


_NC = None


def _get_nc():
    global _NC
    if _NC is None:
        _NC = _build()
    return _NC


def kernel(data1, data2, Wq, bq, Wk, bk):
    data1 = np.asarray(data1, dtype=np.float32)
    data2 = np.asarray(data2, dtype=np.float32)
    Wq = np.ascontiguousarray(np.asarray(Wq, dtype=np.float32))
    bq = np.ascontiguousarray(np.asarray(bq, dtype=np.float32))
    Wk = np.asarray(Wk, dtype=np.float32)
    bk = np.asarray(bk, dtype=np.float32)

    wk_ext = np.zeros((D, D + 1), dtype=np.float32)
    wk_ext[:, :D] = Wk
    bk_ext = np.concatenate([bk, np.ones(1, dtype=np.float32)]).astype(np.float32)

    nc = _get_nc()
    in_maps = [
        {
            "data1": np.ascontiguousarray(data1[b]),
            "data2": np.ascontiguousarray(data2[b]),
            "Wq": Wq,
            "Wk_ext": wk_ext,
            "bq": bq,
            "bk_ext": bk_ext,
        }
        for b in range(B)
    ]
    res = run_bass_kernel_spmd(nc, in_maps, core_ids=list(range(N_CORES)))
    return np.stack([res.results[i]["out"] for i in range(B)], axis=0)


# revision 42
# speedup vs baseline: 1.1706x; 1.1432x over previous
"""CoAttention kernel for Trainium2, data-parallel over batch across 8 NeuronCores.

Per core (one batch element b):
    query = data1[b] @ Wq + bq                      # [2048, 256]
    key   = data2[b] @ Wk + bk                      # [2048, 256]
    attn  = softmax(SCALE * query @ key^T)          # row-constant terms cancel
    out   = attn @ key + query

Device-side strategy:
  - Activations load as fp32 (plain HWDGE DMAs, consolidated), are cast to
    bf16 on DVE, and transposed on the PE (transpose-mode, bf16 PSUM out) so
    the contraction dim lands on partitions.
  - The QT projection runs weights-stationary (bf16, fp32 PSUM accumulate,
    bias via per-partition activation bias); the residual Q [q, d] comes from
    PE transposes of the bf16 QT.
  - scoresT [k, q] orientation lets exp(scoresT) feed the context matmul
    directly as the stationary operand. The scores and context matmuls run in
    fp8e4m3 with DoubleRow packing ([128, 2, n] APs, d = slot*128 + p), which
    is safe because the attention term is tiny relative to the residual.
  - Softmax denominator is a ones-column appended to the key value matrix;
    no max-subtraction (|SCALE*scores| < ~4, exp is well-conditioned there).
  - The q range is processed in two halves, and phase units are interleaved
    in emission order (scores with next-half QT, scores with context) so PE
    matmuls overlap the serial exp work on the scalar engine.
"""

import sys

if "/opt/trn_rl_repo" not in sys.path:
    sys.path.insert(0, "/opt/trn_rl_repo")

from contextlib import ExitStack

import numpy as np

import concourse.bass as bass  # noqa: F401
import concourse.mybir as mybir
import concourse.tile as tile
from concourse import bacc
from concourse.bass_utils import run_bass_kernel_spmd
from concourse.masks import make_identity

B, LQ, LK, DIN, D = 8, 2048, 2048, 1024, 256
N_CORES = 8
SCALE = float(1.0 / np.sqrt(1024.0).astype(np.float32))

BF16 = mybir.dt.bfloat16
FP8 = mybir.dt.float8e4
F32 = mybir.dt.float32
AF = mybir.ActivationFunctionType
PM_DR = mybir.MatmulPerfMode.DoubleRow


def _build():
    nc = bacc.Bacc("TRN2", target_bir_lowering=False, debug=False)
    d1 = nc.dram_tensor("data1", [LQ, DIN], F32, kind="ExternalInput").ap()
    d2 = nc.dram_tensor("data2", [LK, D], F32, kind="ExternalInput").ap()
    wq = nc.dram_tensor("Wq", [DIN, D], F32, kind="ExternalInput").ap()
    wkx = nc.dram_tensor("Wk_ext", [D, D + 1], F32, kind="ExternalInput").ap()
    bq = nc.dram_tensor("bq", [D], F32, kind="ExternalInput").ap()
    bkx = nc.dram_tensor("bk_ext", [D + 1], F32, kind="ExternalInput").ap()
    out = nc.dram_tensor("out", [LQ, D], F32, kind="ExternalOutput").ap()

    QB = LQ // 128  # 16 q blocks
    KB = LK // 128  # 16 k blocks
    IC1 = DIN // 128  # 8
    IC2 = D // 128  # 2

    with tile.TileContext(nc) as tc, ExitStack() as ctx:
        const = ctx.enter_context(tc.tile_pool(name="const", bufs=1))
        big = ctx.enter_context(tc.tile_pool(name="big", bufs=1))
        stage1 = ctx.enter_context(tc.tile_pool(name="stage1", bufs=2))
        stage2 = ctx.enter_context(tc.tile_pool(name="stage2", bufs=2))
        small = ctx.enter_context(tc.tile_pool(name="small", bufs=4))
        ps_gp = ctx.enter_context(tc.tile_pool(name="ps_gp", bufs=2, space="PSUM"))
        ps_sc = ctx.enter_context(tc.tile_pool(name="ps_sc", bufs=2, space="PSUM"))

        # ---------------- constants ----------------
        ones_row = const.tile([1, 128], BF16, tag="ones_row")
        nc.vector.memset(ones_row[:], 1.0)
        ident = const.tile([128, 128], F32, tag="ident")
        make_identity(nc, ident[:])
        ident_bf = const.tile([128, 128], BF16, tag="ident_bf")
        make_identity(nc, ident_bf[:])

        # ---------------- data loads first (sync queue), weights via SWDGE cast ----------------
        d2_st = [stage2.tile([128, 8 * D], F32, tag="d2st", name=f"d2st{g2}")
                 for g2 in range(2)]
        for g2 in range(2):
            for hh in range(2):
                nc.sync.dma_start(
                    out=d2_st[g2][:, hh * 4 * D:(hh + 1) * 4 * D].rearrange(
                        "p (t i) -> p t i", i=D),
                    in_=d2[g2 * 1024 + hh * 512: g2 * 1024 + (hh + 1) * 512, :]
                    .rearrange("(t p) i -> p t i", p=128),
                )
        d1_st = [stage1.tile([128, 4 * DIN], F32, tag="d1st", name=f"d1st{g}")
                 for g in range(4)]
        for g in range(2):
            nc.sync.dma_start(
                out=d1_st[g][:].rearrange("p (t i) -> p t i", i=DIN),
                in_=d1[g * 512:(g + 1) * 512, :].rearrange("(t p) i -> p t i", p=128),
            )
        bq_col = const.tile([128, IC2], F32, tag="bq_col")
        for c in range(IC2):
            nc.sync.dma_start(
                out=bq_col[:, c:c + 1],
                in_=bq[c * 128:(c + 1) * 128].rearrange("(p a) -> p a", a=1),
            )
        for g in range(2, 4):
            nc.sync.dma_start(
                out=d1_st[g][:].rearrange("p (t i) -> p t i", i=DIN),
                in_=d1[g * 512:(g + 1) * 512, :].rearrange("(t p) i -> p t i", p=128),
            )
        wq_sb = const.tile([128, IC1 * D], BF16, tag="wq_sb")
        nc.gpsimd.dma_start(
            out=wq_sb[:].rearrange("p (c d) -> p c d", d=D),
            in_=wq.rearrange("(c p) d -> p c d", p=128),
        )
        wk_sb = const.tile([128, IC2 * (D + 1)], BF16, tag="wk_sb")
        nc.gpsimd.dma_start(
            out=wk_sb[:].rearrange("p (c d) -> p c d", d=D + 1),
            in_=wkx.rearrange("(c p) d -> p c d", p=128),
        )
        wqs = [wq_sb[:, i * D:(i + 1) * D] for i in range(IC1)]
        wks = [wk_sb[:, i * (D + 1):(i + 1) * (D + 1)] for i in range(IC2)]
        bkx_row = const.tile([1, D + 1], BF16, tag="bkx_row")
        nc.gpsimd.dma_start(out=bkx_row[:], in_=bkx.rearrange("(a d) -> a d", a=1))

        # ---------------- data2: 2 group loads -> DVE cast -> PE transposes ----------------
        d2T = [big.tile([128, LK], BF16, tag=f"d2T{i}", name=f"d2T{i}") for i in range(IC2)]
        for g2 in range(2):  # 8 k-tiles per group
            st = d2_st[g2]
            bf = stage2.tile([128, 8 * D], BF16, tag="d2bf", name=f"d2bfs{g2}")
            for hh in range(2):
                nc.vector.tensor_copy(
                    bf[:, hh * 4 * D:(hh + 1) * 4 * D],
                    st[:, hh * 4 * D:(hh + 1) * 4 * D],
                )
            for ic in range(IC2):
                for h in range(2):  # 4 k-tiles per psum batch
                    pt = ps_gp.tile([128, 512], BF16, tag="ps_t",
                                    name=f"pt_d2_{g2}_{ic}_{h}")
                    for j in range(4):
                        kt = 4 * h + j
                        nc.tensor.transpose(
                            pt[:, j * 128:(j + 1) * 128],
                            bf[:, kt * D + ic * 128: kt * D + (ic + 1) * 128],
                            ident_bf[:],
                        )
                    nc.scalar.copy(
                        d2T[ic][:, g2 * 1024 + h * 512: g2 * 1024 + (h + 1) * 512],
                        pt[:],
                    )

        # ---------------- K^T fp8 DoubleRow layout [128, 2, k] (d = dc*128+p) ----------------
        kt_sb = big.tile([128, 2, LK], FP8, tag="kt_sb")
        for dc in range(2):
            for nk in range(LK // 512):
                ps = ps_gp.tile([128, 512], F32, tag="ps_gp")
                for ic in range(IC2):
                    nc.tensor.matmul(
                        ps[:],
                        lhsT=wks[ic][:, dc * 128:(dc + 1) * 128],
                        rhs=d2T[ic][:, nk * 512:(nk + 1) * 512],
                        start=(ic == 0),
                        stop=(ic == IC2 - 1),
                    )
                nc.vector.tensor_copy(kt_sb[:, dc, nk * 512:(nk + 1) * 512], ps[:])

        # ---------------- key value matrix fp8 pairs [128, 2, 257] = [key | 1] ----------------
        key2 = [
            big.tile([128, 2, D + 1], FP8, tag=f"key2_{kp}", name=f"key2_{kp}")
            for kp in range(KB // 2)
        ]
        def key_units():
            return [lambda kb=kb: key_unit(kb) for kb in range(KB)]

        def key_phase():
            for u in key_units():
                u()

        def key_unit(kb):
            if True:
                ps = ps_gp.tile([128, 512], F32, tag="ps_gp")
                p = ps[:, :D + 1]
                for ic in range(IC2):
                    nc.tensor.matmul(
                        p,
                        lhsT=d2T[ic][:, kb * 128:(kb + 1) * 128],
                        rhs=wks[ic],
                        start=(ic == 0),
                        stop=False,
                    )
                nc.tensor.matmul(p, lhsT=ones_row[:], rhs=bkx_row[:], start=False, stop=True)
                nc.vector.tensor_copy(key2[kb // 2][:, kb % 2, :], p)

        # ---------------- data1: 4 group loads -> DVE cast -> PE transposes ----------------
        d1T = [big.tile([128, LQ], BF16, tag=f"d1T{i}", name=f"d1T{i}") for i in range(IC1)]

        def d1_group(g):
            st = d1_st[g]
            bf = stage1.tile([128, 4 * DIN], BF16, tag="d1bf", name=f"d1bfs{g}", bufs=2)
            nc.vector.tensor_copy(bf[:], st[:])
            for ic in range(IC1):
                pt = ps_gp.tile([128, 512], BF16, tag="ps_t", name=f"pt_d1_{g}_{ic}")
                for j in range(4):
                    nc.tensor.transpose(
                        pt[:, j * 128:(j + 1) * 128],
                        bf[:, j * DIN + ic * 128: j * DIN + (ic + 1) * 128],
                        ident_bf[:],
                    )
                nc.vector.tensor_copy(d1T[ic][:, g * 512:(g + 1) * 512], pt[:])

        # only the first half of data1 before QT-h0; groups 2-3 are emitted
        # after the scores-h0 interleave so load waits never block the PE
        # queue ahead of the exp-feeding scores matmuls
        d1_group(0)
        d1_group(1)

        # ---------------- per-half pipeline ----------------
        q_sb = [big.tile([128, D], BF16, tag=f"q{qb}", name=f"q{qb}") for qb in range(QB)]
        qt_sb = big.tile([128, 2, LQ], FP8, tag="qt_sb")
        qtbf = big.tile([128, 2, LQ], BF16, tag="qtbf")
        expT = [
            [big.tile([128, 2, 1024], FP8, tag=f"expT{kp}_{nh}", name=f"expT{kp}_{nh}")
             for nh in range(2)]
            for kp in range(KB // 2)
        ]
        out_sb = [
            big.tile([128, 4 * D], F32, tag=f"out_sb{hq}", name=f"out_sb{hq}")
            for hq in range(4)
        ]

        def qt_units(h):
            units = []
            for dc in range(2):
                for nq in range(h * 2, h * 2 + 2):
                    units.append(lambda dc=dc, nq=nq: qt_unit(dc, nq))
            return units

        def qt_phase(h):
            for u in qt_units(h):
                u()

        def qt_unit(dc, nq):
            if True:
                if True:
                    ps = ps_gp.tile([128, 512], F32, tag="ps_gp")
                    for ic in range(IC1):
                        nc.tensor.matmul(
                            ps[:],
                            lhsT=wqs[ic][:, dc * 128:(dc + 1) * 128],
                            rhs=d1T[ic][:, nq * 512:(nq + 1) * 512],
                            start=(ic == 0),
                            stop=(ic == IC1 - 1),
                        )
                    if nq < 2:
                        # pre-scores window: ACT is idle, use it
                        nc.scalar.activation(
                            qt_sb[:, dc, nq * 512:(nq + 1) * 512], ps[:], AF.Identity,
                            bias=bq_col[:, dc:dc + 1],
                        )
                        nc.scalar.activation(
                            qtbf[:, dc, nq * 512:(nq + 1) * 512], ps[:], AF.Identity,
                            bias=bq_col[:, dc:dc + 1],
                        )
                    else:
                        # interleaved with scores exps: keep off the in-order
                        # ACT queue (PSUM-wait would head-of-line-block exp)
                        nc.vector.tensor_scalar(
                            qt_sb[:, dc, nq * 512:(nq + 1) * 512], ps[:],
                            bq_col[:, dc:dc + 1], None, mybir.AluOpType.add,
                        )
                        nc.vector.tensor_scalar(
                            qtbf[:, dc, nq * 512:(nq + 1) * 512], ps[:],
                            bq_col[:, dc:dc + 1], None, mybir.AluOpType.add,
                        )

        def qres_units(h):
            units = []
            for qg in range(h * 2, h * 2 + 2):
                for dc in range(2):
                    units.append(lambda qg=qg, dc=dc: qres_unit(qg, dc))
            return units

        def qres_phase(h):
            for u in qres_units(h):
                u()

        def qres_unit(qg, dc):
            if True:
                if True:
                    pt = ps_gp.tile([128, 512], BF16, tag="ps_t",
                                    name=f"pt_q_{qg}_{dc}")
                    for j in range(4):
                        qb = qg * 4 + j
                        nc.tensor.transpose(
                            pt[:, j * 128:(j + 1) * 128],
                            qtbf[:, dc, qb * 128:(qb + 1) * 128],
                            ident_bf[:],
                        )
                    for j in range(4):
                        qb = qg * 4 + j
                        nc.vector.tensor_copy(
                            q_sb[qb][:, dc * 128:(dc + 1) * 128],
                            pt[:, j * 128:(j + 1) * 128],
                        )

        def scores_units(nh):
            return [lambda km=km: scores_unit(km, nh) for km in range(KB)]

        def scores_unit(km, nh):
            if True:
                ps = ps_sc.tile([128, 1024], F32, tag="ps_sc")
                for half in range(2):
                    nq = nh * 2 + half
                    nc.tensor.matmul(
                        ps[:, half * 512:(half + 1) * 512],
                        lhsT=kt_sb[:, :, km * 128:(km + 1) * 128],
                        rhs=qt_sb[:, :, nq * 512:(nq + 1) * 512],
                        perf_mode=PM_DR,
                        start=True,
                        stop=True,
                    )
                nc.scalar.activation(
                    expT[km // 2][nh][:, km % 2, :], ps[:], AF.Exp, scale=SCALE
                )

        def ctx_units(h):
            return [lambda qb=qb: ctx_unit(qb) for qb in range(h * 8, h * 8 + 8)]

        def ctx_phase(h):
            for u in ctx_units(h):
                u()

        def ctx_unit(qb):
            if True:
                h, qq = qb // 8, qb % 8
                hq, qqq = qb // 4, qb % 4
                if h == 0:
                    pc_full = ps_gp.tile([128, 512], F32, tag="ps_gp")
                else:
                    # scores pool is idle once scores-h1 is done; borrow it so
                    # the tail context chains don't contend with qres psum use
                    pc_full = ps_sc.tile([128, 512], F32, tag="ps_sc")
                pc = pc_full[:, :D + 1]
                for kp in range(KB // 2):
                    nc.tensor.matmul(
                        pc,
                        lhsT=expT[kp][h][:, :, qq * 128:(qq + 1) * 128],
                        rhs=key2[kp][:],
                        perf_mode=PM_DR,
                        start=(kp == 0),
                        stop=(kp == KB // 2 - 1),
                    )
                rc = small.tile([128, 1], F32, tag="recip")
                nc.vector.reciprocal(rc[:], pc[:, D:D + 1])
                osl = out_sb[hq][:, qqq * D:(qqq + 1) * D]
                nc.vector.tensor_scalar(osl, pc[:, :D], rc[:], None,
                                        mybir.AluOpType.mult)
                nc.vector.tensor_add(osl, osl, q_sb[qb][:])
                if qqq == 3:
                    nc.sync.dma_start(
                        out=out[hq * (LQ // 4):(hq + 1) * (LQ // 4), :].rearrange(
                            "(qt p) d -> p qt d", p=128
                        ),
                        in_=out_sb[hq][:].rearrange("p (qt d) -> p qt d", d=D),
                    )

        def interleave(a, b, ratio):
            a = list(a); b = list(b)
            ia = ib = 0
            while ia < len(a) or ib < len(b):
                for _ in range(ratio):
                    if ia < len(a):
                        a[ia](); ia += 1
                if ib < len(b):
                    b[ib](); ib += 1

        qt_phase(0)
        # scores-h0 interleaved ONLY with early-ready filler (key + qres-h0);
        # anything waiting on late data1 loads would head-of-line block the
        # PE queue ahead of the exp-feeding scores matmuls
        interleave(scores_units(0), key_units() + qres_units(0), 1)
        d1_group(2)
        d1_group(3)
        qt_phase(1)
        # scores-h1 interleaved with ctx-h0
        interleave(scores_units(1), ctx_units(0), 2)
        qr1 = qres_units(1)
        cx1 = ctx_units(1)
        qr1[0](); qr1[1]()
        cx1[0](); cx1[1]()
        qr1[2](); qr1[3]()
        for u in cx1[2:]:
            u()

    nc.compile()
    return nc


_NC = None


def _get_nc():
    global _NC
    if _NC is None:
        _NC = _build()
    return _NC


def kernel(data1, data2, Wq, bq, Wk, bk):
    data1 = np.asarray(data1, dtype=np.float32)
    data2 = np.asarray(data2, dtype=np.float32)
    Wq = np.ascontiguousarray(np.asarray(Wq, dtype=np.float32))
    bq = np.ascontiguousarray(np.asarray(bq, dtype=np.float32))
    Wk = np.asarray(Wk, dtype=np.float32)
    bk = np.asarray(bk, dtype=np.float32)

    wk_ext = np.zeros((D, D + 1), dtype=np.float32)
    wk_ext[:, :D] = Wk
    bk_ext = np.concatenate([bk, np.ones(1, dtype=np.float32)]).astype(np.float32)

    nc = _get_nc()
    in_maps = [
        {
            "data1": np.ascontiguousarray(data1[b]),
            "data2": np.ascontiguousarray(data2[b]),
            "Wq": Wq,
            "Wk_ext": wk_ext,
            "bq": bq,
            "bk_ext": bk_ext,
        }
        for b in range(B)
    ]
    res = run_bass_kernel_spmd(nc, in_maps, core_ids=list(range(N_CORES)))
    return np.stack([res.results[i]["out"] for i in range(B)], axis=0)
